# revision 1
# baseline (speedup 1.0000x reference)
"""ColourCatDSSGINConv on 8 trn2 NeuronCores.

Sharding: nodes are partitioned into 8 contiguous blocks of 6250; each core
aggregates the in-edges of its own nodes (pull model) from a replicated
node-feature table U = [x | c] (48 cols, padded to 64), then runs both GIN
MLP paths on its block.  The x-aggregate is shared across the 4 colour
samples and the mean-path aggregate is derived from the same U-aggregate,
so message passing runs once over 48 columns instead of 96+24.

Aggregation: per-phase (src<32768 / src>=32768, for int16 gather indices)
ELL iterations over degree-sorted nodes; each iteration is a dense-prefix
dma_gather (<=1024 rows/call, padded to a zero table row) followed by DVE
adds into an SBUF accumulator; the two phase accumulators (in different
node orders) are merged into canonical order with dma_scatter_add into a
DRAM table.  BatchNorm is global across cores: one small AllReduce of the
per-feature sums/sumsq.  b1s/b1a cancel inside BatchNorm and are dropped.
Output is written feature-major [320, 6250] per core; the host transposes.
"""
import os
import sys

sys.path.insert(0, "/opt/trn_rl_repo")

import numpy as np

N = 50000
E = 800000
IN = 16
CD = 8
S = 4
EMB = 64
D = IN + CD          # 24
H = 64
BN_EPS = 1e-5

NCORES = 8
P = 128
PC = N // NCORES     # 6250 nodes per core
Q = (PC + P - 1) // P          # 49 column-groups of 128 nodes
SLOTS = Q * P                  # 6272
TROWS = 50176                  # 1 zero row + 50000 nodes + zero pad
LO_ROWS = 32768                # lo window rows [0, 32768): node v at row v+1
HI_BASE = 32768                # hi window: node v (>=32767) at row v+1
HI_ZERO = 50001 - HI_BASE      # a guaranteed-zero row in the hi window
LO_MAX_NODE = 32766
CALL = 1024                    # idxs per dma_gather / dma_scatter_add call
NCHUNK = 512                   # node columns per matmul chunk

last_exec_time_ns = None
_prog_cache = {}
_trace = bool(os.environ.get("GNN_TRACE"))


def _wrap16(flat, pad_val, pad_to):
    """int16 flat token list -> [128, pad_to//16] wrapped+replicated layout
    (token t lives at [t%16, t//16], replicated across the 8 gpsimd groups)."""
    n = len(flat)
    assert pad_to % 16 == 0 and n <= pad_to
    buf = np.full(pad_to, pad_val, dtype=np.int16)
    buf[:n] = flat
    arr = buf.reshape(pad_to // 16, 16).T.copy()
    return np.tile(arr, (8, 1))


def _phase_prep(src, ldst, lo):
    """ELL prep for one (core, phase): degree-desc node order; every edge gets
    (slot t, iteration j, int16 table idx)."""
    deg = np.bincount(ldst, minlength=PC)
    order = np.argsort(-deg, kind="stable")
    rank = np.empty(PC, dtype=np.int64)
    rank[order] = np.arange(PC)
    sidx = np.argsort(ldst, kind="stable")
    sd = ldst[sidx]
    ss = src[sidx]
    starts = np.searchsorted(sd, np.arange(PC))
    j = np.arange(len(sd)) - starts[sd]
    t = rank[sd]
    val = (ss + 1 - (0 if lo else HI_BASE)).astype(np.int16)
    return deg[order], order, t, j, val


def _pieces(n_pad):
    """Per gather-call DVE-add pieces: [(stg_j0, nj, acc_q0), ...] per call."""
    offs = [0]
    for n in n_pad:
        offs.append(offs[-1] + n)
    L = offs[-1]
    Lpad = ((L + CALL - 1) // CALL) * CALL
    out = []
    for c0 in range(0, Lpad, CALL):
        c1 = c0 + CALL
        ps = []
        for j, n in enumerate(n_pad):
            a, b = max(c0, offs[j]), min(c1, offs[j + 1])
            if a < b:
                ps.append(((a - c0) // P, (b - a) // P, (a - offs[j]) // P))
        out.append(ps)
    return out, Lpad


def _build_inputs(x, c, edge_index, W1s, g1s, be1s, W2s, b2s,
                  W1a, g1a, be1a, W2a, b2a):
    src_all = edge_index[0].astype(np.int64)
    dst_all = edge_index[1].astype(np.int64)

    U = np.zeros((TROWS, 64), dtype=np.float32)
    U[1:N + 1, :IN] = x
    U[1:N + 1, IN:48] = c.reshape(N, S * CD)

    core_of = dst_all // PC
    meta = {}
    idx_arrays = {}
    scat_arrays = {}
    per = {}
    for k in range(NCORES):
        m = core_of == k
        s_k = src_all[m]
        d_k = dst_all[m] % PC
        lo_m = s_k <= LO_MAX_NODE
        per[(k, "lo")] = _phase_prep(s_k[lo_m], d_k[lo_m], True)
        per[(k, "hi")] = _phase_prep(s_k[~lo_m], d_k[~lo_m], False)

    for ph in ("lo", "hi"):
        maxdeg = max(int(per[(k, ph)][0][0]) if per[(k, ph)][0].size else 0
                     for k in range(NCORES))
        n_pad = []
        for j in range(maxdeg):
            nj = max(int(np.sum(per[(k, ph)][0] > j)) for k in range(NCORES))
            n_pad.append(((nj + P - 1) // P) * P)
        offs = np.concatenate([[0], np.cumsum(n_pad)]).astype(np.int64)
        _, Lpad = _pieces(n_pad)
        zero_idx = 0 if ph == "lo" else HI_ZERO
        meta[ph] = tuple(n_pad)
        for k in range(NCORES):
            _deg, order, t, j, val = per[(k, ph)]
            flat = np.full(Lpad, zero_idx, dtype=np.int16)
            flat[offs[j] + t] = val
            idx_arrays[(k, ph)] = _wrap16(flat, zero_idx, Lpad)
    # canonical per-core slot order = lo-phase order; only hi needs a merge
    orders = {}
    for k in range(NCORES):
        order_lo = per[(k, "lo")][1]
        rank_lo = np.empty(PC, dtype=np.int64)
        rank_lo[order_lo] = np.arange(PC)
        order_hi = per[(k, "hi")][1]
        ids = np.concatenate([rank_lo[order_hi],
                              np.arange(PC, SLOTS)]).astype(np.int16)
        scat_arrays[(k, "hi")] = _wrap16(ids, 0, SLOTS)
        orders[k] = order_lo

    import ml_dtypes
    wall = np.zeros((128, 320), dtype=np.float32)
    for s in range(S):
        wall[0:IN, s * H:(s + 1) * H] = W1s[0:IN, :]
        wall[IN + CD * s:IN + CD * (s + 1), s * H:(s + 1) * H] = W1s[IN:D, :]
    wall[64:64 + IN, 256:320] = W1a[0:IN, :]
    wall[64 + IN:64 + D, 256:320] = W1a[IN:D, :]
    w2 = np.concatenate([W2s, W2a], axis=1).astype(ml_dtypes.bfloat16)

    bnc = np.zeros((64, 6), dtype=np.float32)
    bnc[:, 0] = g1s
    bnc[:, 1] = be1s
    bnc[:, 2] = 1.0 / (N * S)
    bnc[:, 3] = g1a
    bnc[:, 4] = be1a
    bnc[:, 5] = 1.0 / N
    b2v = (b2s + b2a).astype(np.float32).reshape(64, 1)
    wallT = np.zeros((64, 640), dtype=np.float32)
    for b in range(5):
        wallT[:, b * 128:(b + 1) * 128] = wall[:, b * 64:(b + 1) * 64].T

    in_maps = []
    for k in range(NCORES):
        uo = np.zeros((SLOTS, 64), dtype=np.float32)
        uo[:PC] = U[1 + k * PC + orders[k]]
        uo_t = uo.reshape(Q, P, 64).transpose(1, 0, 2).reshape(P, Q * 64).copy()
        in_maps.append({
            "utab": U,
            "uown": uo_t,
            "ilo": idx_arrays[(k, "lo")],
            "ihi": idx_arrays[(k, "hi")],
            "shi": scat_arrays[(k, "hi")],
            "wall": wall.astype(ml_dtypes.bfloat16),
            "w2": w2,
            "bnc": bnc,
            "b2v": b2v,
            "wallT": wallT,
            "wallf": wall,
        })
    return in_maps, meta, orders


def _build_program(meta, eps_s, eps_a, clo, chi):
    import concourse.bacc as bacc
    import concourse.tile as tile
    import concourse.mybir as mybir
    from concourse.masks import make_identity

    f32 = mybir.dt.float32
    bf16 = mybir.dt.bfloat16
    i16 = mybir.dt.int16
    add = mybir.AluOpType.add
    sub = mybir.AluOpType.subtract
    mult = mybir.AluOpType.mult
    Relu = mybir.ActivationFunctionType.Relu
    Square = mybir.ActivationFunctionType.Square

    nc = bacc.Bacc("TRN2", target_bir_lowering=False, debug=False,
                   num_devices=NCORES)
    utab = nc.dram_tensor("utab", [TROWS, 64], f32, kind="ExternalInput").ap()
    uown = nc.dram_tensor("uown", [P, Q * 64], f32, kind="ExternalInput").ap()
    ilo = nc.dram_tensor("ilo", [128, clo // 16], i16, kind="ExternalInput").ap()
    ihi = nc.dram_tensor("ihi", [128, chi // 16], i16, kind="ExternalInput").ap()
    shi = nc.dram_tensor("shi", [128, SLOTS // 16], i16, kind="ExternalInput").ap()
    wallp = nc.dram_tensor("wall", [128, 320], bf16, kind="ExternalInput").ap()
    w2p = nc.dram_tensor("w2", [64, 128], bf16, kind="ExternalInput").ap()
    bncp = nc.dram_tensor("bnc", [64, 6], f32, kind="ExternalInput").ap()
    b2vp = nc.dram_tensor("b2v", [64, 1], f32, kind="ExternalInput").ap()
    wallTp = nc.dram_tensor("wallT", [64, 640], f32, kind="ExternalInput").ap()
    wallfp = nc.dram_tensor("wallf", [128, 320], f32, kind="ExternalInput").ap()
    o_ap = nc.dram_tensor("o", [320, PC], f32, kind="ExternalOutput").ap()

    nck = [(c0, min(NCHUNK, PC - c0)) for c0 in range(0, PC, NCHUNK)]

    with tile.TileContext(nc) as tc:
        with (
            tc.tile_pool(name="dram", bufs=1, space="DRAM") as dp,
            tc.tile_pool(name="persist", bufs=1) as mp,
        ):
            accd = dp.tile([SLOTS, 64], f32)

            # ---------- gather + ELL accumulate; hi merged via scatter ----------
            acc_lo = mp.tile([P, Q, 64], f32, tag="acc_lo")
            nc.vector.memset(acc_lo[:].rearrange("p q d -> p (q d)"), 0.0)
            with tc.tile_pool(name="gath", bufs=1) as gp, \
                 tc.tile_pool(name="stg", bufs=4) as sp:
                zt = gp.tile([P, Q * 64], f32, tag="zt")
                nc.vector.memset(zt[:], 0.0)
                nc.sync.dma_start(
                    out=accd[:].rearrange("(q p) d -> p q d", p=P),
                    in_=zt[:].rearrange("p (q d) -> p q d", d=64))
                for ph, idx_ap, cph in (("hi", ihi, chi), ("lo", ilo, clo)):
                    pieces, _ = _pieces(list(meta[ph]))
                    it = gp.tile([128, cph // 16], i16, tag=f"it_{ph}",
                                 name=f"it_{ph}")
                    nc.sync.dma_start(out=it[:], in_=idx_ap[:])
                    if ph == "lo":
                        acc = acc_lo
                    else:
                        acc = gp.tile([P, Q, 64], f32, tag="acc_hi",
                                      name="acc_hi")
                        nc.vector.memset(acc[:].rearrange("p q d -> p (q d)"),
                                         0.0)
                    win = (utab[0:LO_ROWS, :] if ph == "lo"
                           else utab[HI_BASE:TROWS, :])
                    for ci, ps in enumerate(pieces):
                        g = sp.tile([P, CALL // P, 64], f32, tag="stg",
                                    name=f"g_{ph}_{ci}")
                        nc.gpsimd.dma_gather(
                            out_ap=g[:], in_ap=win,
                            idxs_ap=it[:, ci * (CALL // 16):
                                       (ci + 1) * (CALL // 16)],
                            num_idxs=CALL, num_idxs_reg=CALL, elem_size=64)
                        for (sj, nj, qa) in ps:
                            nc.vector.tensor_tensor(
                                out=acc[:, qa:qa + nj, :],
                                in0=acc[:, qa:qa + nj, :],
                                in1=g[:, sj:sj + nj, :], op=add)
                    if ph == "hi":
                        st = gp.tile([128, SLOTS // 16], i16, tag="st_hi",
                                     name="st_hi")
                        nc.sync.dma_start(out=st[:], in_=shi[:])
                        for c0 in range(0, SLOTS, CALL):
                            n = min(CALL, SLOTS - c0)
                            nc.gpsimd.dma_scatter_add(
                                accd[:], acc[:, c0 // P:(c0 + n) // P, :],
                                st[:, c0 // 16:(c0 + n) // 16],
                                num_idxs=n, num_idxs_reg=n, elem_size=64)

            # ---------- stacked = [(1+es)own+agg (64) | m_a (24) | 0 ] ----------
            stackedT = mp.tile([P, SLOTS], bf16, tag="stackedT")
            stk2 = mp.tile([P, Q, 128], f32, tag="stk2")
            stk2b = mp.tile([P, Q, 128], bf16, tag="stk2b")
            with tc.tile_pool(name="build", bufs=1) as bp, \
                 tc.tile_pool(name="pst", bufs=2, space="PSUM") as pst:
                accm = bp.tile([P, Q, 64], f32, tag="accm")
                nc.sync.dma_start(
                    out=accm[:], in_=accd[:].rearrange("(q p) d -> p q d", p=P))
                own = bp.tile([P, Q, 64], f32, tag="own")
                nc.sync.dma_start(
                    out=own[:].rearrange("p q d -> p (q d)"), in_=uown[:])
                stk = stk2
                nc.vector.memset(stk[:].rearrange("p q d -> p (q d)"), 0.0)
                # shared path: (1+es)*own + agg   (cols 0:64)
                nc.vector.tensor_scalar_mul(
                    out=stk[:, :, 0:64], in0=own[:], scalar1=float(1.0 + eps_s))
                nc.vector.tensor_tensor(
                    out=stk[:, :, 0:64], in0=stk[:, :, 0:64], in1=accm[:], op=add)
                nc.vector.tensor_tensor(
                    out=stk[:, :, 0:64], in0=stk[:, :, 0:64], in1=acc_lo[:],
                    op=add)
                # agg path x part (cols 64:80)
                nc.vector.tensor_scalar_mul(
                    out=stk[:, :, 64:80], in0=own[:, :, 0:IN],
                    scalar1=float(1.0 + eps_a))
                nc.vector.tensor_tensor(
                    out=stk[:, :, 64:80], in0=stk[:, :, 64:80],
                    in1=accm[:, :, 0:IN], op=add)
                nc.vector.tensor_tensor(
                    out=stk[:, :, 64:80], in0=stk[:, :, 64:80],
                    in1=acc_lo[:, :, 0:IN], op=add)
                # agg path c part (cols 80:88): mean_s((1+ea)c_s + agg_c_s)
                tcc = bp.tile([P, Q, 32], f32, tag="tcc")
                nc.vector.tensor_scalar_mul(
                    out=tcc[:], in0=own[:, :, IN:48], scalar1=float(1.0 + eps_a))
                nc.vector.tensor_tensor(
                    out=tcc[:], in0=tcc[:], in1=accm[:, :, IN:48], op=add)
                nc.vector.tensor_tensor(
                    out=tcc[:], in0=tcc[:], in1=acc_lo[:, :, IN:48], op=add)
                nc.vector.tensor_tensor(
                    out=stk[:, :, 80:88], in0=tcc[:, :, 0:8],
                    in1=tcc[:, :, 8:16], op=add)
                nc.vector.tensor_tensor(
                    out=stk[:, :, 80:88], in0=stk[:, :, 80:88],
                    in1=tcc[:, :, 16:24], op=add)
                nc.vector.tensor_tensor(
                    out=stk[:, :, 80:88], in0=stk[:, :, 80:88],
                    in1=tcc[:, :, 24:32], op=add)
                nc.vector.tensor_scalar_mul(
                    out=stk[:, :, 80:88], in0=stk[:, :, 80:88], scalar1=0.25)
                # bf16 copy once, then transpose to feature-major
                nc.vector.tensor_copy(
                    out=stk2b[:].rearrange("p q d -> p (q d)"),
                    in_=stk[:].rearrange("p q d -> p (q d)"))
                ident = bp.tile([P, P], bf16, tag="ident")
                make_identity(nc, ident[:])
                for q0 in range(0, Q, 4):
                    qn = min(4, Q - q0)
                    tp = pst.tile([P, 4, P], bf16, tag="tp", name=f"tp_{q0}",
                                  space="PSUM")
                    for qq in range(qn):
                        nc.tensor.transpose(out=tp[:, qq, :],
                                            in_=stk2b[:, q0 + qq, :],
                                            identity=ident[:])
                    nc.vector.tensor_copy(
                        out=stackedT[:, q0 * P:(q0 + qn) * P],
                        in_=tp[:, 0:qn, :].rearrange("p q d -> p (q d)"))

            # ---------- BN stats via second moments ----------
            wt = mp.tile([P, 320], bf16, tag="wt")
            nc.sync.dma_start(out=wt[:], in_=wallp[:])
            wtf = mp.tile([P, 320], f32, tag="wtf")
            nc.sync.dma_start(out=wtf[:], in_=wallfp[:])
            wtT = mp.tile([64, 640], f32, tag="wtT")
            nc.sync.dma_start(out=wtT[:], in_=wallTp[:])
            s1 = mp.tile([64, 2], f32, tag="s1")
            s2 = mp.tile([64, 2], f32, tag="s2")
            nc.vector.memset(s1[:], 0.0)
            nc.vector.memset(s2[:], 0.0)
            m2s = mp.tile([P, P], f32, tag="m2s")
            colsum = mp.tile([P, 1], f32, tag="colsum")
            with tc.tile_pool(name="pm2m", bufs=1, space="PSUM") as pm2m, \
                 tc.tile_pool(name="stat", bufs=4) as stp:
                m2 = pm2m.tile([P, P], f32, tag="m2", space="PSUM")
                for q in range(Q):
                    nc.tensor.matmul(out=m2[:], lhsT=stk2b[:, q, :],
                                     rhs=stk2b[:, q, :],
                                     start=(q == 0), stop=(q == Q - 1))
                nc.vector.tensor_copy(out=m2s[:], in_=m2[:])
                nc.vector.tensor_reduce(out=colsum[:], in_=stackedT[:, 0:PC],
                                        axis=mybir.AxisListType.X, op=add)
                for b in range(5):
                    col = 0 if b < 4 else 1
                    wm = pm2m.tile([64, P], f32, tag="wm", name=f"wm_{b}",
                                   space="PSUM")
                    nc.tensor.matmul(out=wm[:], lhsT=wtf[:, b * 64:(b + 1) * 64],
                                     rhs=m2s[:], start=True, stop=True)
                    prod = stp.tile([64, P], f32, tag="prod", name=f"prod_{b}")
                    nc.vector.tensor_tensor(
                        out=prod[:], in0=wm[:],
                        in1=wtT[:, b * 128:(b + 1) * 128], op=mult)
                    r2 = stp.tile([64, 1], f32, tag="r2", name=f"r2_{b}")
                    nc.vector.tensor_reduce(out=r2[:], in_=prod[:],
                                            axis=mybir.AxisListType.X, op=add)
                    nc.vector.tensor_tensor(
                        out=s2[:, col:col + 1], in0=s2[:, col:col + 1],
                        in1=r2[:], op=add)
                    p1 = pm2m.tile([64, 1], f32, tag="p1", name=f"p1_{b}",
                                   space="PSUM")
                    nc.tensor.matmul(out=p1[:], lhsT=wtf[:, b * 64:(b + 1) * 64],
                                     rhs=colsum[:], start=True, stop=True)
                    r1 = stp.tile([64, 1], f32, tag="r1", name=f"r1_{b}")
                    nc.vector.tensor_copy(out=r1[:], in_=p1[:])
                    nc.vector.tensor_tensor(
                        out=s1[:, col:col + 1], in0=s1[:, col:col + 1],
                        in1=r1[:], op=add)

            # ---------- AllReduce of stats ----------
            cin = dp.tile([64, 4], f32)
            cout = dp.tile([64, 4], f32)
            stats = mp.tile([64, 4], f32, tag="stats")
            nc.vector.tensor_copy(out=stats[:, 0:2], in_=s1[:])
            nc.vector.tensor_copy(out=stats[:, 2:4], in_=s2[:])
            nc.gpsimd.dma_start(out=cin[:], in_=stats[:])
            nc.gpsimd.collective_compute(
                "AllReduce", add,
                replica_groups=[list(range(NCORES))],
                ins=[cin.opt()], outs=[cout.opt()])
            nc.gpsimd.dma_start(out=stats[:], in_=cout[:])

            # ---------- BN scale/bias ----------
            bnct = mp.tile([64, 6], f32, tag="bnct")
            nc.sync.dma_start(out=bnct[:], in_=bncp[:])
            gcols = mp.tile([64, 2], f32, tag="gcols")
            becols = mp.tile([64, 2], f32, tag="becols")
            incols = mp.tile([64, 2], f32, tag="incols")
            nc.vector.tensor_copy(out=gcols[:, 0:1], in_=bnct[:, 0:1])
            nc.vector.tensor_copy(out=gcols[:, 1:2], in_=bnct[:, 3:4])
            nc.vector.tensor_copy(out=becols[:, 0:1], in_=bnct[:, 1:2])
            nc.vector.tensor_copy(out=becols[:, 1:2], in_=bnct[:, 4:5])
            nc.vector.tensor_copy(out=incols[:, 0:1], in_=bnct[:, 2:3])
            nc.vector.tensor_copy(out=incols[:, 1:2], in_=bnct[:, 5:6])
            mu = mp.tile([64, 2], f32, tag="mu")
            var = mp.tile([64, 2], f32, tag="var")
            scl = mp.tile([64, 2], f32, tag="scl")
            bia = mp.tile([64, 2], f32, tag="bia")
            nc.vector.tensor_tensor(out=mu[:], in0=stats[:, 0:2],
                                    in1=incols[:], op=mult)
            nc.vector.tensor_tensor(out=var[:], in0=stats[:, 2:4],
                                    in1=incols[:], op=mult)
            nc.vector.tensor_tensor(out=scl[:], in0=mu[:], in1=mu[:], op=mult)
            nc.vector.tensor_tensor(out=var[:], in0=var[:], in1=scl[:], op=sub)
            nc.vector.tensor_scalar_add(out=var[:], in0=var[:],
                                        scalar1=float(BN_EPS))
            nc.scalar.sqrt(out=var[:], in_=var[:])
            nc.vector.reciprocal(out=var[:], in_=var[:])
            nc.vector.tensor_tensor(out=scl[:], in0=var[:], in1=gcols[:],
                                    op=mult)
            nc.vector.tensor_tensor(out=bia[:], in0=mu[:], in1=scl[:], op=mult)
            nc.vector.tensor_tensor(out=bia[:], in0=becols[:], in1=bia[:],
                                    op=sub)

            # ---------- pass 2: recompute W1, BN+relu, W2, output ----------
            w2t = mp.tile([64, 128], bf16, tag="w2t")
            nc.sync.dma_start(out=w2t[:], in_=w2p[:])
            b2t = mp.tile([64, 1], f32, tag="b2t")
            nc.sync.dma_start(out=b2t[:], in_=b2vp[:])
            with tc.tile_pool(name="pm2", bufs=2, space="PSUM") as pm2, \
                 tc.tile_pool(name="po", bufs=1, space="PSUM") as po, \
                 tc.tile_pool(name="rts", bufs=2) as rts, \
                 tc.tile_pool(name="outs", bufs=3) as osb:
                for ci, (c0, cn) in enumerate(nck):
                    psC = pm2.tile([64, NCHUNK], f32, tag="mC",
                                   name=f"mC_{ci}", space="PSUM")
                    nc.tensor.matmul(
                        out=psC[:, :cn], lhsT=wt[:, 256:320],
                        rhs=stackedT[:, c0:c0 + cn], start=True, stop=True)
                    psAB = []
                    for half in range(2):
                        ps = pm2.tile([128, NCHUNK], f32, tag=f"mA_{half}",
                                      name=f"mA_{half}_{ci}", space="PSUM")
                        nc.tensor.matmul(
                            out=ps[:, :cn],
                            lhsT=wt[:, half * 128:(half + 1) * 128],
                            rhs=stackedT[:, c0:c0 + cn], start=True, stop=True)
                        psAB.append(ps)
                    rt = {}
                    for b in (4, 0, 1, 2, 3):
                        col = 0 if b < 4 else 1
                        if b < 4:
                            src = psAB[b // 2][(b % 2) * 64:(b % 2 + 1) * 64]
                        else:
                            src = psC[:]
                        r = rts.tile([64, NCHUNK], bf16, tag=f"rt_{b}",
                                     name=f"rt_{b}_{ci}")
                        nc.scalar.activation(
                            out=r[:, :cn], in_=src[:, :cn], func=Relu,
                            bias=bia[:, col:col + 1], scale=scl[:, col:col + 1])
                        rt[b] = r
                    pag = po.tile([64, NCHUNK], f32, tag="pag",
                                  name=f"pag_{ci}", space="PSUM")
                    nc.tensor.matmul(
                        out=pag[:, :cn], lhsT=w2t[:, 64:128],
                        rhs=rt[4][:, 0:cn], start=True, stop=True)
                    aggsb = osb.tile([64, NCHUNK], f32, tag="aggsb",
                                     name=f"aggsb_{ci}")
                    nc.vector.tensor_tensor(
                        out=aggsb[:, :cn], in0=pag[:, :cn],
                        in1=b2t[:, 0:1].to_broadcast([64, cn]), op=add)
                    for s in range(S):
                        pso = po.tile([64, NCHUNK], f32, tag="po",
                                      name=f"po_{ci}_{s}", space="PSUM")
                        nc.tensor.matmul(
                            out=pso[:, :cn], lhsT=w2t[:, 0:64],
                            rhs=rt[s][:, 0:cn], start=True, stop=True)
                        ot = osb.tile([64, NCHUNK], f32, tag="ot",
                                      name=f"ot_{ci}_{s}")
                        nc.vector.tensor_tensor(
                            out=ot[:, :cn], in0=pso[:, :cn],
                            in1=aggsb[:, :cn], op=add)
                        nc.sync.dma_start(
                            out=o_ap[s * 64:(s + 1) * 64, c0:c0 + cn],
                            in_=ot[:, :cn])
    nc.compile()
    return nc


def kernel(**inputs):
    global last_exec_time_ns
    from concourse import bass_utils

    x = np.asarray(inputs["x"], np.float32)
    c = np.asarray(inputs["c"], np.float32)
    edge_index = np.asarray(inputs["edge_index"])
    eps_s = float(np.asarray(inputs["eps_shared"]))
    eps_a = float(np.asarray(inputs["eps_agg"]))
    args = [np.asarray(inputs[k], np.float32) for k in
            ("W1s", "g1s", "be1s", "W2s", "b2s",
             "W1a", "g1a", "be1a", "W2a", "b2a")]

    in_maps, meta, orders = _build_inputs(x, c, edge_index, *args)
    clo = in_maps[0]["ilo"].shape[1] * 16
    chi = in_maps[0]["ihi"].shape[1] * 16
    key = (meta["lo"], meta["hi"], eps_s, eps_a, clo, chi)
    if key not in _prog_cache:
        _prog_cache[key] = _build_program(meta, eps_s, eps_a, clo, chi)
    nc = _prog_cache[key]

    kwargs = {}
    if _trace:
        try:
            import axon_profile_shim  # noqa: F401
        except ImportError:
            pass
        kwargs["trace"] = True
    res = bass_utils.run_bass_kernel_spmd(
        nc, in_maps, core_ids=list(range(NCORES)), **kwargs)
    last_exec_time_ns = res.exec_time_ns

    out = np.empty((N, S * EMB), dtype=np.float32)
    for k in range(NCORES):
        ok = res.results[k]["o"]          # [320, PC]; rows 256:320 unused
        out[k * PC + orders[k]] = ok[:256].T
    return out



# revision 5
# speedup vs baseline: 2.5532x; 2.5532x over previous
"""ColourCatDSSGINConv on 8 trn2 NeuronCores.

Sharding: nodes are partitioned into 8 contiguous blocks of 6250; each core
aggregates the in-edges of its own nodes (pull model) from a replicated
node-feature table U = [x | c] (48 cols, padded to 64), then runs both GIN
MLP paths on its block.  The x-aggregate is shared across the 4 colour
samples and the mean-path aggregate is derived from the same U-aggregate,
so message passing runs once over 48 columns instead of 96+24.

Aggregation: per-phase (src<32768 / src>=32768, for int16 gather indices)
ELL iterations over degree-sorted nodes; each iteration is a dense-prefix
dma_gather (<=1024 rows/call, padded to a zero table row) followed by DVE
adds into an SBUF accumulator; the two phase accumulators (in different
node orders) are merged into canonical order with dma_scatter_add into a
DRAM table.  BatchNorm is global across cores: one small AllReduce of the
per-feature sums/sumsq.  b1s/b1a cancel inside BatchNorm and are dropped.
Output is written feature-major [320, 6250] per core; the host transposes.
"""
import os
import sys

sys.path.insert(0, "/opt/trn_rl_repo")

import numpy as np

N = 50000
E = 800000
IN = 16
CD = 8
S = 4
EMB = 64
D = IN + CD          # 24
H = 64
BN_EPS = 1e-5

NCORES = 8
P = 128
PC = N // NCORES     # 6250 nodes per core
Q = (PC + P - 1) // P          # 49 column-groups of 128 nodes
SLOTS = Q * P                  # 6272
TROWS = 50176                  # 1 zero row + 50000 nodes + zero pad
LO_ROWS = 32768                # lo window rows [0, 32768): node v at row v+1
HI_BASE = 32768                # hi window: node v (>=32767) at row v+1
HI_ZERO = 50001 - HI_BASE      # a guaranteed-zero row in the hi window
LO_MAX_NODE = 32766
CALL = 1024                    # idxs per dma_gather / dma_scatter_add call
NCHUNK = 512                   # node columns per matmul chunk

last_exec_time_ns = None
_prog_cache = {}
_trace = bool(os.environ.get("GNN_TRACE"))


def _wrap16(flat, pad_val, pad_to):
    """int16 flat token list -> [128, pad_to//16] wrapped+replicated layout
    (token t lives at [t%16, t//16], replicated across the 8 gpsimd groups)."""
    n = len(flat)
    assert pad_to % 16 == 0 and n <= pad_to
    buf = np.full(pad_to, pad_val, dtype=np.int16)
    buf[:n] = flat
    arr = buf.reshape(pad_to // 16, 16).T.copy()
    return np.tile(arr, (8, 1))


def _phase_prep(src, ldst, lo):
    """ELL prep for one (core, phase): degree-desc node order; every edge gets
    (slot t, iteration j, int16 table idx)."""
    deg = np.bincount(ldst, minlength=PC)
    order = np.argsort(-deg, kind="stable")
    rank = np.empty(PC, dtype=np.int64)
    rank[order] = np.arange(PC)
    sidx = np.argsort(ldst, kind="stable")
    sd = ldst[sidx]
    ss = src[sidx]
    starts = np.searchsorted(sd, np.arange(PC))
    j = np.arange(len(sd)) - starts[sd]
    t = rank[sd]
    val = (ss + 1 - (0 if lo else HI_BASE)).astype(np.int16)
    return deg[order], order, t, j, val


def _pieces(n_pad):
    """Per gather-call DVE-add pieces: [(stg_j0, nj, acc_q0), ...] per call."""
    offs = [0]
    for n in n_pad:
        offs.append(offs[-1] + n)
    L = offs[-1]
    Lpad = ((L + CALL - 1) // CALL) * CALL
    out = []
    for c0 in range(0, Lpad, CALL):
        c1 = c0 + CALL
        ps = []
        for j, n in enumerate(n_pad):
            a, b = max(c0, offs[j]), min(c1, offs[j + 1])
            if a < b:
                ps.append(((a - c0) // P, (b - a) // P, (a - offs[j]) // P))
        out.append(ps)
    return out, Lpad


def _build_inputs(x, c, edge_index, W1s, g1s, be1s, W2s, b2s,
                  W1a, g1a, be1a, W2a, b2a):
    src_all = edge_index[0].astype(np.int64)
    dst_all = edge_index[1].astype(np.int64)

    U = np.zeros((TROWS, 64), dtype=np.float32)
    U[1:N + 1, :IN] = x
    U[1:N + 1, IN:48] = c.reshape(N, S * CD)

    core_of = dst_all // PC
    meta = {}
    idx_arrays = {}
    scat_arrays = {}
    per = {}
    for k in range(NCORES):
        m = core_of == k
        s_k = src_all[m]
        d_k = dst_all[m] % PC
        lo_m = s_k <= LO_MAX_NODE
        per[(k, "lo")] = _phase_prep(s_k[lo_m], d_k[lo_m], True)
        per[(k, "hi")] = _phase_prep(s_k[~lo_m], d_k[~lo_m], False)

    for ph in ("lo", "hi"):
        maxdeg = max(int(per[(k, ph)][0][0]) if per[(k, ph)][0].size else 0
                     for k in range(NCORES))
        n_pad = []
        for j in range(maxdeg):
            nj = max(int(np.sum(per[(k, ph)][0] > j)) for k in range(NCORES))
            n_pad.append(((nj + P - 1) // P) * P)
        offs = np.concatenate([[0], np.cumsum(n_pad)]).astype(np.int64)
        _, Lpad = _pieces(n_pad)
        zero_idx = 0 if ph == "lo" else HI_ZERO
        meta[ph] = tuple(n_pad)
        for k in range(NCORES):
            _deg, order, t, j, val = per[(k, ph)]
            flat = np.full(Lpad, zero_idx, dtype=np.int16)
            flat[offs[j] + t] = val
            idx_arrays[(k, ph)] = _wrap16(flat, zero_idx, Lpad)
    # canonical per-core slot order = lo-phase order; only hi needs a merge
    orders = {}
    for k in range(NCORES):
        order_lo = per[(k, "lo")][1]
        rank_lo = np.empty(PC, dtype=np.int64)
        rank_lo[order_lo] = np.arange(PC)
        order_hi = per[(k, "hi")][1]
        ids = np.concatenate([rank_lo[order_hi],
                              np.arange(PC, SLOTS)]).astype(np.int16)
        scat_arrays[(k, "hi")] = _wrap16(ids, 0, SLOTS)
        orders[k] = order_lo

    import ml_dtypes
    wall = np.zeros((128, 320), dtype=np.float32)
    for s in range(S):
        wall[0:IN, s * H:(s + 1) * H] = W1s[0:IN, :]
        wall[IN + CD * s:IN + CD * (s + 1), s * H:(s + 1) * H] = W1s[IN:D, :]
    wall[64:64 + IN, 256:320] = W1a[0:IN, :]
    wall[64 + IN:64 + D, 256:320] = W1a[IN:D, :]
    w2 = np.concatenate([W2s, W2a], axis=1).astype(ml_dtypes.bfloat16)

    bnc = np.zeros((64, 6), dtype=np.float32)
    bnc[:, 0] = g1s
    bnc[:, 1] = be1s
    bnc[:, 2] = 1.0 / (N * S)
    bnc[:, 3] = g1a
    bnc[:, 4] = be1a
    bnc[:, 5] = 1.0 / N
    b2v = (b2s + b2a).astype(np.float32).reshape(64, 1)
    wallT = np.zeros((64, 640), dtype=np.float32)
    for b in range(5):
        wallT[:, b * 128:(b + 1) * 128] = wall[:, b * 64:(b + 1) * 64].T

    in_maps = []
    for k in range(NCORES):
        uo = np.zeros((SLOTS, 64), dtype=np.float32)
        uo[:PC] = U[1 + k * PC + orders[k]]
        uo_t = uo.reshape(Q, P, 64).transpose(1, 0, 2).reshape(P, Q * 64).copy()
        in_maps.append({
            "utab": U,
            "uown": uo_t,
            "ilo": idx_arrays[(k, "lo")],
            "ihi": idx_arrays[(k, "hi")],
            "shi": scat_arrays[(k, "hi")],
            "wall": wall.astype(ml_dtypes.bfloat16),
            "w2": w2,
            "bnc": bnc,
            "b2v": b2v,
            "wallT": wallT,
            "wallf": wall,
        })
    return in_maps, meta, orders


def _build_program(meta, eps_s, eps_a, clo, chi):
    import concourse.bacc as bacc
    import concourse.tile as tile
    import concourse.mybir as mybir
    from concourse.masks import make_identity

    f32 = mybir.dt.float32
    bf16 = mybir.dt.bfloat16
    i16 = mybir.dt.int16
    add = mybir.AluOpType.add
    sub = mybir.AluOpType.subtract
    mult = mybir.AluOpType.mult
    Relu = mybir.ActivationFunctionType.Relu
    Square = mybir.ActivationFunctionType.Square

    nc = bacc.Bacc("TRN2", target_bir_lowering=False, debug=False,
                   num_devices=NCORES, num_swdge_queues=4)
    utab = nc.dram_tensor("utab", [TROWS, 64], f32, kind="ExternalInput").ap()
    uown = nc.dram_tensor("uown", [P, Q * 64], f32, kind="ExternalInput").ap()
    ilo = nc.dram_tensor("ilo", [128, clo // 16], i16, kind="ExternalInput").ap()
    ihi = nc.dram_tensor("ihi", [128, chi // 16], i16, kind="ExternalInput").ap()
    shi = nc.dram_tensor("shi", [128, SLOTS // 16], i16, kind="ExternalInput").ap()
    wallp = nc.dram_tensor("wall", [128, 320], bf16, kind="ExternalInput").ap()
    w2p = nc.dram_tensor("w2", [64, 128], bf16, kind="ExternalInput").ap()
    bncp = nc.dram_tensor("bnc", [64, 6], f32, kind="ExternalInput").ap()
    b2vp = nc.dram_tensor("b2v", [64, 1], f32, kind="ExternalInput").ap()
    wallTp = nc.dram_tensor("wallT", [64, 640], f32, kind="ExternalInput").ap()
    wallfp = nc.dram_tensor("wallf", [128, 320], f32, kind="ExternalInput").ap()
    o_ap = nc.dram_tensor("o", [320, PC], f32, kind="ExternalOutput").ap()

    nck = [(c0, min(NCHUNK, PC - c0)) for c0 in range(0, PC, NCHUNK)]

    with tile.TileContext(nc) as tc:
        with (
            tc.tile_pool(name="dram", bufs=1, space="DRAM") as dp,
            tc.tile_pool(name="persist", bufs=1) as mp,
        ):
            accd = dp.tile([SLOTS, 64], f32)

            # ---------- gather + ELL accumulate; hi merged via scatter ----------
            acc_lo = mp.tile([P, Q, 64], f32, tag="acc_lo")
            nc.vector.memset(acc_lo[:].rearrange("p q d -> p (q d)"), 0.0)
            with tc.tile_pool(name="gath", bufs=1) as gp, \
                 tc.tile_pool(name="stg", bufs=8) as sp:
                zt = gp.tile([P, Q * 64], f32, tag="zt")
                nc.vector.memset(zt[:], 0.0)
                nc.sync.dma_start(
                    out=accd[:].rearrange("(q p) d -> p q d", p=P),
                    in_=zt[:].rearrange("p (q d) -> p q d", d=64))
                for ph, idx_ap, cph in (("hi", ihi, chi), ("lo", ilo, clo)):
                    pieces, _ = _pieces(list(meta[ph]))
                    it = gp.tile([128, cph // 16], i16, tag=f"it_{ph}",
                                 name=f"it_{ph}")
                    nc.sync.dma_start(out=it[:], in_=idx_ap[:])
                    if ph == "lo":
                        acc = acc_lo
                    else:
                        acc = gp.tile([P, Q, 64], f32, tag="acc_hi",
                                      name="acc_hi")
                        nc.vector.memset(acc[:].rearrange("p q d -> p (q d)"),
                                         0.0)
                    win = (utab[0:LO_ROWS, :] if ph == "lo"
                           else utab[HI_BASE:TROWS, :])
                    for ci, ps in enumerate(pieces):
                        g = sp.tile([P, CALL // P, 64], f32, tag="stg",
                                    name=f"g_{ph}_{ci}")
                        nc.gpsimd.dma_gather(
                            out_ap=g[:], in_ap=win,
                            idxs_ap=it[:, ci * (CALL // 16):
                                       (ci + 1) * (CALL // 16)],
                            num_idxs=CALL, num_idxs_reg=CALL, elem_size=64,
                            queue_num=ci % 4)
                        for (sj, nj, qa) in ps:
                            nc.vector.tensor_tensor(
                                out=acc[:, qa:qa + nj, :],
                                in0=acc[:, qa:qa + nj, :],
                                in1=g[:, sj:sj + nj, :], op=add)
                    if ph == "hi":
                        st = gp.tile([128, SLOTS // 16], i16, tag="st_hi",
                                     name="st_hi")
                        nc.sync.dma_start(out=st[:], in_=shi[:])
                        for qi, c0 in enumerate(range(0, SLOTS, CALL)):
                            n = min(CALL, SLOTS - c0)
                            nc.gpsimd.dma_scatter_add(
                                accd[:], acc[:, c0 // P:(c0 + n) // P, :],
                                st[:, c0 // 16:(c0 + n) // 16],
                                num_idxs=n, num_idxs_reg=n, elem_size=64,
                                queue_num=qi % 4)

            # ---------- stacked = [(1+es)own+agg (64) | m_a (24) | 0 ] ----------
            stackedT = mp.tile([P, SLOTS], bf16, tag="stackedT")
            stk2 = mp.tile([P, Q, 128], f32, tag="stk2")
            stk2b = mp.tile([P, Q, 128], bf16, tag="stk2b")
            with tc.tile_pool(name="build", bufs=1) as bp, \
                 tc.tile_pool(name="pst", bufs=2, space="PSUM") as pst:
                accm = bp.tile([P, Q, 64], f32, tag="accm")
                nc.sync.dma_start(
                    out=accm[:], in_=accd[:].rearrange("(q p) d -> p q d", p=P))
                own = bp.tile([P, Q, 64], f32, tag="own")
                nc.sync.dma_start(
                    out=own[:].rearrange("p q d -> p (q d)"), in_=uown[:])
                stk = stk2
                nc.vector.memset(stk[:].rearrange("p q d -> p (q d)"), 0.0)
                # shared path: (1+es)*own + agg   (cols 0:64)
                nc.vector.tensor_scalar_mul(
                    out=stk[:, :, 0:64], in0=own[:], scalar1=float(1.0 + eps_s))
                nc.vector.tensor_tensor(
                    out=stk[:, :, 0:64], in0=stk[:, :, 0:64], in1=accm[:], op=add)
                nc.vector.tensor_tensor(
                    out=stk[:, :, 0:64], in0=stk[:, :, 0:64], in1=acc_lo[:],
                    op=add)
                # agg path x part (cols 64:80)
                nc.vector.tensor_scalar_mul(
                    out=stk[:, :, 64:80], in0=own[:, :, 0:IN],
                    scalar1=float(1.0 + eps_a))
                nc.vector.tensor_tensor(
                    out=stk[:, :, 64:80], in0=stk[:, :, 64:80],
                    in1=accm[:, :, 0:IN], op=add)
                nc.vector.tensor_tensor(
                    out=stk[:, :, 64:80], in0=stk[:, :, 64:80],
                    in1=acc_lo[:, :, 0:IN], op=add)
                # agg path c part (cols 80:88): mean_s((1+ea)c_s + agg_c_s)
                tcc = bp.tile([P, Q, 32], f32, tag="tcc")
                nc.vector.tensor_scalar_mul(
                    out=tcc[:], in0=own[:, :, IN:48], scalar1=float(1.0 + eps_a))
                nc.vector.tensor_tensor(
                    out=tcc[:], in0=tcc[:], in1=accm[:, :, IN:48], op=add)
                nc.vector.tensor_tensor(
                    out=tcc[:], in0=tcc[:], in1=acc_lo[:, :, IN:48], op=add)
                nc.vector.tensor_tensor(
                    out=stk[:, :, 80:88], in0=tcc[:, :, 0:8],
                    in1=tcc[:, :, 8:16], op=add)
                nc.vector.tensor_tensor(
                    out=stk[:, :, 80:88], in0=stk[:, :, 80:88],
                    in1=tcc[:, :, 16:24], op=add)
                nc.vector.tensor_tensor(
                    out=stk[:, :, 80:88], in0=stk[:, :, 80:88],
                    in1=tcc[:, :, 24:32], op=add)
                nc.vector.tensor_scalar_mul(
                    out=stk[:, :, 80:88], in0=stk[:, :, 80:88], scalar1=0.25)
                # bf16 copy once, then transpose to feature-major
                nc.vector.tensor_copy(
                    out=stk2b[:].rearrange("p q d -> p (q d)"),
                    in_=stk[:].rearrange("p q d -> p (q d)"))
                ident = bp.tile([P, P], bf16, tag="ident")
                make_identity(nc, ident[:])
                for q0 in range(0, Q, 4):
                    qn = min(4, Q - q0)
                    tp = pst.tile([P, 4, P], bf16, tag="tp", name=f"tp_{q0}",
                                  space="PSUM")
                    for qq in range(qn):
                        nc.tensor.transpose(out=tp[:, qq, :],
                                            in_=stk2b[:, q0 + qq, :],
                                            identity=ident[:])
                    nc.vector.tensor_copy(
                        out=stackedT[:, q0 * P:(q0 + qn) * P],
                        in_=tp[:, 0:qn, :].rearrange("p q d -> p (q d)"))

            # ---------- BN stats via second moments ----------
            wt = mp.tile([P, 320], bf16, tag="wt")
            nc.sync.dma_start(out=wt[:], in_=wallp[:])
            wtf = mp.tile([P, 320], f32, tag="wtf")
            nc.sync.dma_start(out=wtf[:], in_=wallfp[:])
            wtT = mp.tile([64, 640], f32, tag="wtT")
            nc.sync.dma_start(out=wtT[:], in_=wallTp[:])
            s1 = mp.tile([64, 2], f32, tag="s1")
            s2 = mp.tile([64, 2], f32, tag="s2")
            nc.vector.memset(s1[:], 0.0)
            nc.vector.memset(s2[:], 0.0)
            m2s = mp.tile([P, P], f32, tag="m2s")
            colsum = mp.tile([P, 1], f32, tag="colsum")
            with tc.tile_pool(name="pm2m", bufs=1, space="PSUM") as pm2m, \
                 tc.tile_pool(name="stat", bufs=4) as stp:
                m2 = pm2m.tile([P, P], f32, tag="m2", space="PSUM")
                for q in range(Q):
                    nc.tensor.matmul(out=m2[:], lhsT=stk2b[:, q, :],
                                     rhs=stk2b[:, q, :],
                                     start=(q == 0), stop=(q == Q - 1))
                nc.vector.tensor_copy(out=m2s[:], in_=m2[:])
                nc.vector.tensor_reduce(out=colsum[:], in_=stackedT[:, 0:PC],
                                        axis=mybir.AxisListType.X, op=add)
                for b in range(5):
                    col = 0 if b < 4 else 1
                    wm = pm2m.tile([64, P], f32, tag="wm", name=f"wm_{b}",
                                   space="PSUM")
                    nc.tensor.matmul(out=wm[:], lhsT=wtf[:, b * 64:(b + 1) * 64],
                                     rhs=m2s[:], start=True, stop=True)
                    prod = stp.tile([64, P], f32, tag="prod", name=f"prod_{b}")
                    nc.vector.tensor_tensor(
                        out=prod[:], in0=wm[:],
                        in1=wtT[:, b * 128:(b + 1) * 128], op=mult)
                    r2 = stp.tile([64, 1], f32, tag="r2", name=f"r2_{b}")
                    nc.vector.tensor_reduce(out=r2[:], in_=prod[:],
                                            axis=mybir.AxisListType.X, op=add)
                    nc.vector.tensor_tensor(
                        out=s2[:, col:col + 1], in0=s2[:, col:col + 1],
                        in1=r2[:], op=add)
                    p1 = pm2m.tile([64, 1], f32, tag="p1", name=f"p1_{b}",
                                   space="PSUM")
                    nc.tensor.matmul(out=p1[:], lhsT=wtf[:, b * 64:(b + 1) * 64],
                                     rhs=colsum[:], start=True, stop=True)
                    r1 = stp.tile([64, 1], f32, tag="r1", name=f"r1_{b}")
                    nc.vector.tensor_copy(out=r1[:], in_=p1[:])
                    nc.vector.tensor_tensor(
                        out=s1[:, col:col + 1], in0=s1[:, col:col + 1],
                        in1=r1[:], op=add)

            # ---------- AllReduce of stats ----------
            cin = dp.tile([64, 4], f32)
            cout = dp.tile([64, 4], f32)
            stats = mp.tile([64, 4], f32, tag="stats")
            nc.vector.tensor_copy(out=stats[:, 0:2], in_=s1[:])
            nc.vector.tensor_copy(out=stats[:, 2:4], in_=s2[:])
            nc.gpsimd.dma_start(out=cin[:], in_=stats[:])
            nc.gpsimd.collective_compute(
                "AllReduce", add,
                replica_groups=[list(range(NCORES))],
                ins=[cin.opt()], outs=[cout.opt()])
            nc.gpsimd.dma_start(out=stats[:], in_=cout[:])

            # ---------- BN scale/bias ----------
            bnct = mp.tile([64, 6], f32, tag="bnct")
            nc.sync.dma_start(out=bnct[:], in_=bncp[:])
            gcols = mp.tile([64, 2], f32, tag="gcols")
            becols = mp.tile([64, 2], f32, tag="becols")
            incols = mp.tile([64, 2], f32, tag="incols")
            nc.vector.tensor_copy(out=gcols[:, 0:1], in_=bnct[:, 0:1])
            nc.vector.tensor_copy(out=gcols[:, 1:2], in_=bnct[:, 3:4])
            nc.vector.tensor_copy(out=becols[:, 0:1], in_=bnct[:, 1:2])
            nc.vector.tensor_copy(out=becols[:, 1:2], in_=bnct[:, 4:5])
            nc.vector.tensor_copy(out=incols[:, 0:1], in_=bnct[:, 2:3])
            nc.vector.tensor_copy(out=incols[:, 1:2], in_=bnct[:, 5:6])
            mu = mp.tile([64, 2], f32, tag="mu")
            var = mp.tile([64, 2], f32, tag="var")
            scl = mp.tile([64, 2], f32, tag="scl")
            bia = mp.tile([64, 2], f32, tag="bia")
            nc.vector.tensor_tensor(out=mu[:], in0=stats[:, 0:2],
                                    in1=incols[:], op=mult)
            nc.vector.tensor_tensor(out=var[:], in0=stats[:, 2:4],
                                    in1=incols[:], op=mult)
            nc.vector.tensor_tensor(out=scl[:], in0=mu[:], in1=mu[:], op=mult)
            nc.vector.tensor_tensor(out=var[:], in0=var[:], in1=scl[:], op=sub)
            nc.vector.tensor_scalar_add(out=var[:], in0=var[:],
                                        scalar1=float(BN_EPS))
            nc.scalar.sqrt(out=var[:], in_=var[:])
            nc.vector.reciprocal(out=var[:], in_=var[:])
            nc.vector.tensor_tensor(out=scl[:], in0=var[:], in1=gcols[:],
                                    op=mult)
            nc.vector.tensor_tensor(out=bia[:], in0=mu[:], in1=scl[:], op=mult)
            nc.vector.tensor_tensor(out=bia[:], in0=becols[:], in1=bia[:],
                                    op=sub)

            # ---------- pass 2: recompute W1, BN+relu, W2, output ----------
            w2t = mp.tile([64, 128], bf16, tag="w2t")
            nc.sync.dma_start(out=w2t[:], in_=w2p[:])
            b2t = mp.tile([64, 1], f32, tag="b2t")
            nc.sync.dma_start(out=b2t[:], in_=b2vp[:])
            with tc.tile_pool(name="pm2", bufs=2, space="PSUM") as pm2, \
                 tc.tile_pool(name="po", bufs=1, space="PSUM") as po, \
                 tc.tile_pool(name="rts", bufs=2) as rts, \
                 tc.tile_pool(name="outs", bufs=3) as osb:
                for ci, (c0, cn) in enumerate(nck):
                    psC = pm2.tile([64, NCHUNK], f32, tag="mC",
                                   name=f"mC_{ci}", space="PSUM")
                    nc.tensor.matmul(
                        out=psC[:, :cn], lhsT=wt[:, 256:320],
                        rhs=stackedT[:, c0:c0 + cn], start=True, stop=True)
                    psAB = []
                    for half in range(2):
                        ps = pm2.tile([128, NCHUNK], f32, tag=f"mA_{half}",
                                      name=f"mA_{half}_{ci}", space="PSUM")
                        nc.tensor.matmul(
                            out=ps[:, :cn],
                            lhsT=wt[:, half * 128:(half + 1) * 128],
                            rhs=stackedT[:, c0:c0 + cn], start=True, stop=True)
                        psAB.append(ps)
                    rt = {}
                    for b in (4, 0, 1, 2, 3):
                        col = 0 if b < 4 else 1
                        if b < 4:
                            src = psAB[b // 2][(b % 2) * 64:(b % 2 + 1) * 64]
                        else:
                            src = psC[:]
                        r = rts.tile([64, NCHUNK], bf16, tag=f"rt_{b}",
                                     name=f"rt_{b}_{ci}")
                        nc.scalar.activation(
                            out=r[:, :cn], in_=src[:, :cn], func=Relu,
                            bias=bia[:, col:col + 1], scale=scl[:, col:col + 1])
                        rt[b] = r
                    pag = po.tile([64, NCHUNK], f32, tag="pag",
                                  name=f"pag_{ci}", space="PSUM")
                    nc.tensor.matmul(
                        out=pag[:, :cn], lhsT=w2t[:, 64:128],
                        rhs=rt[4][:, 0:cn], start=True, stop=True)
                    aggsb = osb.tile([64, NCHUNK], f32, tag="aggsb",
                                     name=f"aggsb_{ci}")
                    nc.vector.tensor_tensor(
                        out=aggsb[:, :cn], in0=pag[:, :cn],
                        in1=b2t[:, 0:1].to_broadcast([64, cn]), op=add)
                    for s in range(S):
                        pso = po.tile([64, NCHUNK], f32, tag="po",
                                      name=f"po_{ci}_{s}", space="PSUM")
                        nc.tensor.matmul(
                            out=pso[:, :cn], lhsT=w2t[:, 0:64],
                            rhs=rt[s][:, 0:cn], start=True, stop=True)
                        ot = osb.tile([64, NCHUNK], f32, tag="ot",
                                      name=f"ot_{ci}_{s}")
                        nc.vector.tensor_tensor(
                            out=ot[:, :cn], in0=pso[:, :cn],
                            in1=aggsb[:, :cn], op=add)
                        nc.sync.dma_start(
                            out=o_ap[s * 64:(s + 1) * 64, c0:c0 + cn],
                            in_=ot[:, :cn])
    nc.compile()
    return nc


def kernel(**inputs):
    global last_exec_time_ns
    from concourse import bass_utils

    x = np.asarray(inputs["x"], np.float32)
    c = np.asarray(inputs["c"], np.float32)
    edge_index = np.asarray(inputs["edge_index"])
    eps_s = float(np.asarray(inputs["eps_shared"]))
    eps_a = float(np.asarray(inputs["eps_agg"]))
    args = [np.asarray(inputs[k], np.float32) for k in
            ("W1s", "g1s", "be1s", "W2s", "b2s",
             "W1a", "g1a", "be1a", "W2a", "b2a")]

    in_maps, meta, orders = _build_inputs(x, c, edge_index, *args)
    clo = in_maps[0]["ilo"].shape[1] * 16
    chi = in_maps[0]["ihi"].shape[1] * 16
    key = (meta["lo"], meta["hi"], eps_s, eps_a, clo, chi)
    if key not in _prog_cache:
        _prog_cache[key] = _build_program(meta, eps_s, eps_a, clo, chi)
    nc = _prog_cache[key]

    kwargs = {}
    if _trace:
        try:
            import axon_profile_shim  # noqa: F401
        except ImportError:
            pass
        kwargs["trace"] = True
    res = bass_utils.run_bass_kernel_spmd(
        nc, in_maps, core_ids=list(range(NCORES)), **kwargs)
    last_exec_time_ns = res.exec_time_ns

    out = np.empty((N, S * EMB), dtype=np.float32)
    for k in range(NCORES):
        ok = res.results[k]["o"]          # [320, PC]; rows 256:320 unused
        out[k * PC + orders[k]] = ok[:256].T
    return out

